# revision 7
# baseline (speedup 1.0000x reference)
"""Trainium2 Bass kernel for batched multi-head attention with per-batch mask.

Reference semantics (B=8, H=16, S=1024, D=64):
    scores = Q @ K^T                      # [B,H,S,S]
    scores = where(mask[b], -1e9, scores) # per-batch mask
    attn   = softmax(scores / sqrt(D))
    out    = attn @ V

Key observations used here:
  * A masked batch (mask[b]=True) has constant scores, so softmax is exactly
    uniform (1/S) and out[b,h,q,:] = mean_k V[b,h,k,:].  That degenerate case
    is computed directly on host; only unmasked (b,h) units go to the device.
  * For unmasked units |scores/8| <= ~7, so softmax without max-subtraction
    (exp(x)/sum exp(x)) is numerically safe and mathematically identical.
  * bf16 operands keep rel-err ~8e-3 (tolerance 2e-2) while streaming the
    PE at 1 col/cycle and halving HBM traffic vs fp32.
  * The softmax division runs on HOST: the device ships unnormalized out^T
    plus the denominator row ([65, S] per unit), deleting the device-side
    tail chain (reciprocal, broadcast, multiply).
  * All inputs for one unit live in ONE packed DRAM tensor so each unit is
    a single contiguous DMA (DMA_DIRECT2D issue costs ~650ns on the sync
    queue; 5 DMAs/unit serialized the head).  Unit 0 is split in 3 pieces
    ordered so mm1 can start after the first ~60%.

Packed per-unit layout [128, 2560] bf16 (5KB/partition):
  cols    0:1024  qt   Q^T duplicated on partitions 64..127 (row-group rhs)
  cols 1024:1536  kt   K^T packed: rows 0:64 = even chunks, 64:128 = odd
                       (chunk pair p at cols 1024+128p : 1024+128(p+1))
  cols 1536:2560  vx   [p, c, m]: vx[p,c,0:64]=V[c*128+p,:], vx[p,c,64:]=1

Device algorithm per unit (S=1024 split into 8 chunks of 128 along k):
  mm1:  T[k,q]  = sum_d K[k,d]*Q[q,d]       K=64 matmuls at half rate, so
        chunk pairs pack into row groups (0,0)/(64,0) and run concurrently.
  exp:  E[k,q]  = exp(T/8)                  (ScalarE, PSUM->SBUF, bf16 out)
  mm2:  U[m,q] += sum_k Vx[k,m]*E[k,q]      (lhsT = Vx chunk = [V | ones])
        -> U[0:64,q] = unnormalized out^T, U[64,q] = softmax denominator
  out:  copy U[0:65] to SBUF (releases PSUM), DMA to HBM (last unit split
        in halves so the final transfer is short).
"""

import numpy as np

B, H, S, D = 8, 16, 1024, 64
P = 128                      # SBUF partitions / k-chunk size
NCHUNK = S // P              # 8 k-chunks per unit
NHALF = 2                    # PSUM bank limit: matmul N<=512 fp32 out
NCORES = 8

QT0 = 0                      # packed-layout column offsets
KT0 = 1024
VX0 = 1536
TW = 2560

_program_cache = {}


def _build_program(n_units):
    import concourse.mybir as mybir
    import concourse.tile as tile
    from concourse import bacc

    f32 = mybir.dt.float32
    bf16 = mybir.dt.bfloat16
    nc = bacc.Bacc("TRN2", target_bir_lowering=False, debug=False)

    in_d = nc.dram_tensor("inp", [n_units, P, TW], bf16, kind="ExternalInput").ap()
    out_d = nc.dram_tensor("out", [n_units, D + 1, S], f32, kind="ExternalOutput").ap()

    with tile.TileContext(nc) as tc:
        with (
            tc.tile_pool(name="ip", bufs=3) as ip,
            tc.tile_pool(name="ep", bufs=4) as ep,
            tc.tile_pool(name="rp", bufs=2) as rp,
            tc.tile_pool(name="wp", bufs=1) as wpool,
            tc.tile_pool(name="pt", bufs=3, space="PSUM") as pt,   # 3 x 2 banks
            tc.tile_pool(name="pu", bufs=1, space="PSUM") as pu,   # 2 banks
        ):
            # Preload the exp table set (~2.7us) while the first DMA is in
            # flight: walrus inserts the ACT_TABLE_LOAD before the first
            # ACTIVATE in program order, so a junk exp up front moves the
            # load off the critical path.
            dummy = wpool.tile([1, 16], f32, name="dummy_act")
            nc.vector.memset(dummy, 0.0)
            dummy_o = wpool.tile([1, 16], bf16, name="dummy_act_o")
            nc.scalar.activation(
                dummy_o, dummy, mybir.ActivationFunctionType.Exp,
                bias=0.0, scale=1.0,
            )

            # PE warmup: a few junk matmuls issued before the first real
            # matmul start the HAM activity window while unit 0's DMA is in
            # flight.  Kept short so they don't delay the first real mm1.
            wk = wpool.tile([P, 512], bf16)
            nc.vector.memset(wk, 0.0)
            w_ps = pt.tile([P, S], f32, tag="tps", name="warm")
            for i in range(4):
                nc.tensor.matmul(
                    w_ps[:, 0:512], lhsT=wk[:, 0:P], rhs=wk,
                    start=True, stop=True,
                )

            # Unit 0's input split across BOTH HWDGE rings so the issue costs
            # (~650ns each) overlap: qt+kt (needed first, for mm1) on the
            # sync ring; vx (needed only after the first exp) on the scalar
            # ring, where it queues behind the dummy exp.
            tiles_in = [ip.tile([P, TW], bf16, name=f"in{j}") for j in range(n_units)]
            nc.sync.dma_start(tiles_in[0][:, 0:VX0], in_d[0][:, 0:VX0])
            nc.scalar.dma_start(tiles_in[0][:, VX0:TW], in_d[0][:, VX0:TW])

            for j in range(n_units):
                t = tiles_in[j]
                # prefetch the next unit's input ahead of this unit's store
                # on the in-order sync queue.
                if j + 1 < n_units:
                    nc.sync.dma_start(tiles_in[j + 1], in_d[j + 1])

                u_ps = pu.tile([P, S], f32)

                def mm1_pair(cp, t=t):
                    ks = slice(KT0 + cp * P, KT0 + (cp + 1) * P)
                    ta = pt.tile([P, S], f32, tag="tps", name=f"ta{cp}")
                    tb = pt.tile([P, S], f32, tag="tps", name=f"tb{cp}")
                    for h in range(NHALF):
                        qs = slice(h * 512, (h + 1) * 512)
                        nc.tensor.matmul(
                            ta[:, qs],
                            lhsT=t[0:D, ks],
                            rhs=t[0:D, qs],
                            start=True, stop=True,
                            tile_position=(0, 0),
                        )
                        nc.tensor.matmul(
                            tb[:, qs],
                            lhsT=t[D:P, ks],
                            rhs=t[D:P, qs],
                            start=True, stop=True,
                            tile_position=(64, 0),
                        )
                    return ta, tb

                def vxc(c, t=t):
                    return t[:, VX0 + c * P:VX0 + (c + 1) * P]

                # software-pipelined chunk loop: the next pair's mm1s are
                # emitted (adjacently, for row-group packing) before this
                # pair's exp+mm2 consumers.
                NP = NCHUNK // 2
                tiles = mm1_pair(0)
                for cp in range(NP):
                    nxt = mm1_pair(cp + 1) if cp + 1 < NP else None
                    for c, t_ps in zip((2 * cp, 2 * cp + 1), tiles):
                        e_sb = ep.tile([P, S], bf16)
                        nc.scalar.activation(
                            e_sb, t_ps, mybir.ActivationFunctionType.Exp,
                            bias=0.0, scale=0.125,
                        )
                        for h in range(NHALF):
                            qs = slice(h * 512, (h + 1) * 512)
                            nc.tensor.matmul(
                                u_ps[:, qs],
                                lhsT=vxc(c),
                                rhs=e_sb[:, qs],
                                start=(c == 0),
                                stop=(c == NCHUNK - 1),
                            )
                    tiles = nxt

                # U rows 0..63 hold out^T, row 64 the softmax denominator.
                # Copy U out of PSUM immediately (releases u_ps for the next
                # unit), then ship it; host does the division.  The last
                # unit streams in halves so the tail is copy/2 + dma/2.
                u_sb = rp.tile([D + 1, S], f32)
                if j == n_units - 1:
                    # Split the last store so the halves issue on both HWDGE
                    # rings in parallel and the final transfer is short.
                    qs0, qs1 = slice(0, 512), slice(512, S)
                    nc.vector.tensor_copy(out=u_sb[:, qs0], in_=u_ps[0:D + 1, qs0])
                    nc.scalar.dma_start(out_d[j][:, qs0], u_sb[:, qs0])
                    nc.vector.tensor_copy(out=u_sb[:, qs1], in_=u_ps[0:D + 1, qs1])
                    nc.sync.dma_start(out_d[j][:, qs1], u_sb[:, qs1])
                else:
                    nc.vector.tensor_copy(out=u_sb, in_=u_ps[0:D + 1, :])
                    nc.sync.dma_start(out_d[j], u_sb)
    nc.compile()
    return nc


def _get_program(n_units):
    if n_units not in _program_cache:
        _program_cache[n_units] = _build_program(n_units)
    return _program_cache[n_units]


def _pack_unit(dst, qt, kt, vr):
    """Fill one packed [128, TW] bf16 slab from QT/KT [64, S] and V [S, 64]."""
    dst[0:D, QT0:QT0 + S] = qt
    dst[D:P, QT0:QT0 + S] = qt
    # kt packed: chunk pair p -> cols 128p:128(p+1), even chunk rows 0:64,
    # odd chunk rows 64:128
    kc = kt.reshape(D, NCHUNK, P)                         # [d, c, k]
    kp = dst[:, KT0:KT0 + 512].reshape(P, NCHUNK // 2, P)
    kp[0:D] = kc[:, 0::2]
    kp[D:P] = kc[:, 1::2]
    vx = dst[:, VX0:TW].reshape(P, NCHUNK, P)
    vx[:, :, 0:D] = vr.reshape(NCHUNK, P, D).transpose(1, 0, 2)
    vx[:, :, D:P] = np.float32(1.0)


def _prepare(Q, K, V, mask):
    """Host-side sharding. Returns (out_skeleton, units_per_core, in_maps)."""
    import ml_dtypes

    bf16 = ml_dtypes.bfloat16
    Q = np.ascontiguousarray(Q, dtype=np.float32)
    K = np.ascontiguousarray(K, dtype=np.float32)
    V = np.ascontiguousarray(V, dtype=np.float32)
    mask_b = np.asarray(mask).reshape(B).astype(bool)

    out = np.empty((B, H, S, D), dtype=np.float32)

    # Masked batches: softmax over a constant row is exactly uniform -> mean of V.
    for b in np.nonzero(mask_b)[0]:
        mv = V[b].mean(axis=1, dtype=np.float32)          # [H, D]
        out[b] = np.broadcast_to(mv[:, None, :], (H, S, D))

    units = [(b, h) for b in range(B) if not mask_b[b] for h in range(H)]
    if not units:
        return out, None, None

    # Pad to a multiple of NCORES with duplicates (identical redundant work).
    n_per = -(-len(units) // NCORES)
    padded = units + [units[0]] * (n_per * NCORES - len(units))
    per_core = [padded[i::NCORES] for i in range(NCORES)]

    QT = Q.transpose(0, 1, 3, 2).astype(bf16)             # [B,H,D,S]
    KT = K.transpose(0, 1, 3, 2).astype(bf16)
    Vr = V.astype(bf16)

    in_maps = []
    for core_units in per_core:
        inp = np.empty((len(core_units), P, TW), bf16)
        for s, (b, h) in enumerate(core_units):
            _pack_unit(inp[s], QT[b, h], KT[b, h], Vr[b, h])
        in_maps.append({"inp": inp})
    return out, per_core, in_maps


def _run_device(n_units, in_maps, trace=False, trace_cores=None):
    from concourse import bass_utils

    nc = _get_program(n_units)
    return bass_utils.run_bass_kernel_spmd(
        nc,
        in_maps,
        list(range(NCORES)),
        trace=trace,
        trace_cores=trace_cores,
    )


def kernel(Q, K, V, mask, _trace=False, _result_box=None):
    out, per_core, in_maps = _prepare(Q, K, V, mask)
    if in_maps is None:
        return out
    res = _run_device(len(per_core[0]), in_maps, trace=_trace)
    if _result_box is not None:
        _result_box.append(res)
    for i, core_units in enumerate(per_core):
        core_out = res.results[i]["out"]                  # [n, D+1, S]
        for s, (b, h) in enumerate(core_units):
            u = np.asarray(core_out[s], dtype=np.float32)
            out[b, h] = (u[0:D] / u[D:D + 1]).T
    return out


# revision 10
# speedup vs baseline: 1.0199x; 1.0199x over previous
"""Trainium2 Bass kernel for batched multi-head attention with per-batch mask.

Reference semantics (B=8, H=16, S=1024, D=64):
    scores = Q @ K^T                      # [B,H,S,S]
    scores = where(mask[b], -1e9, scores) # per-batch mask
    attn   = softmax(scores / sqrt(D))
    out    = attn @ V

Key observations used here:
  * A masked batch (mask[b]=True) has constant scores, so softmax is exactly
    uniform (1/S) and out[b,h,q,:] = mean_k V[b,h,k,:].  That degenerate case
    is computed directly on host; only unmasked (b,h) units go to the device.
  * For unmasked units |scores/8| <= ~7, so softmax without max-subtraction
    (exp(x)/sum exp(x)) is numerically safe and mathematically identical.
  * bf16 operands keep rel-err ~8e-3 (tolerance 2e-2) while streaming the
    PE at 1 col/cycle and halving HBM traffic vs fp32.
  * The softmax division runs on HOST: the device ships unnormalized out^T
    plus the denominator row ([65, S] per unit), deleting the device-side
    tail chain (reciprocal, broadcast, multiply).
  * All inputs for one unit live in ONE packed DRAM tensor so each unit is
    a single contiguous DMA (DMA_DIRECT2D issue costs ~650ns on the sync
    queue; 5 DMAs/unit serialized the head).  Unit 0 is split in 3 pieces
    ordered so mm1 can start after the first ~60%.

Packed per-unit layout [128, 2560] bf16 (5KB/partition):
  cols    0:1024  qt   Q^T duplicated on partitions 64..127 (row-group rhs)
  cols 1024:1536  kt   K^T packed: rows 0:64 = even chunks, 64:128 = odd
                       (chunk pair p at cols 1024+128p : 1024+128(p+1))
  cols 1536:2560  vx   [p, c, m]: vx[p,c,0:64]=V[c*128+p,:], vx[p,c,64:]=1

Device algorithm per unit (S=1024 split into 8 chunks of 128 along k):
  mm1:  T[k,q]  = sum_d K[k,d]*Q[q,d]       K=64 matmuls at half rate, so
        chunk pairs pack into row groups (0,0)/(64,0) and run concurrently.
  exp:  E[k,q]  = exp(T/8)                  (ScalarE, PSUM->SBUF, bf16 out)
  mm2:  U[m,q] += sum_k Vx[k,m]*E[k,q]      (lhsT = Vx chunk = [V | ones])
        -> U[0:64,q] = unnormalized out^T, U[64,q] = softmax denominator
  out:  copy U[0:65] to SBUF (releases PSUM), DMA to HBM (last unit split
        in halves so the final transfer is short).
"""

import numpy as np

B, H, S, D = 8, 16, 1024, 64
P = 128                      # SBUF partitions / k-chunk size
NCHUNK = S // P              # 8 k-chunks per unit
NHALF = 2                    # PSUM bank limit: matmul N<=512 fp32 out
NCORES = 8

QT0 = 0                      # packed-layout column offsets
KT0 = 1024
VX0 = 1536
TW = 2560

_program_cache = {}


def _build_program(n_units):
    import concourse.mybir as mybir
    import concourse.tile as tile
    from concourse import bacc

    f32 = mybir.dt.float32
    bf16 = mybir.dt.bfloat16
    nc = bacc.Bacc("TRN2", target_bir_lowering=False, debug=False)

    in_d = nc.dram_tensor("inp", [n_units, P, TW], bf16, kind="ExternalInput").ap()
    out_d = nc.dram_tensor("out", [n_units, D + 1, S], bf16, kind="ExternalOutput").ap()

    with tile.TileContext(nc) as tc:
        with (
            tc.tile_pool(name="ip", bufs=3) as ip,
            tc.tile_pool(name="ep", bufs=4) as ep,
            tc.tile_pool(name="rp", bufs=2) as rp,
            tc.tile_pool(name="wp", bufs=1) as wpool,
            tc.tile_pool(name="pt", bufs=3, space="PSUM") as pt,   # 3 x 2 banks
            tc.tile_pool(name="pu", bufs=1, space="PSUM") as pu,   # 2 banks
        ):
            # Preload the exp table set (~2.7us) while the first DMA is in
            # flight: walrus inserts the ACT_TABLE_LOAD before the first
            # ACTIVATE in program order, so a junk exp up front moves the
            # load off the critical path.
            dummy = wpool.tile([1, 16], f32, name="dummy_act")
            nc.vector.memset(dummy, 0.0)
            dummy_o = wpool.tile([1, 16], bf16, name="dummy_act_o")
            nc.scalar.activation(
                dummy_o, dummy, mybir.ActivationFunctionType.Exp,
                bias=0.0, scale=1.0,
            )

            # PE warmup: a few junk matmuls issued before the first real
            # matmul start the HAM activity window while unit 0's DMA is in
            # flight.  Kept short so they don't delay the first real mm1.
            wk = wpool.tile([P, 512], bf16)
            nc.vector.memset(wk, 0.0)
            w_ps = pt.tile([P, S], f32, tag="tps", name="warm")
            for i in range(10):
                nc.tensor.matmul(
                    w_ps[:, 0:512], lhsT=wk[:, 0:P], rhs=wk,
                    start=True, stop=True,
                )

            # Unit 0's input split across BOTH HWDGE rings so the issue costs
            # (~650ns each) overlap: qt+kt (needed first, for mm1) on the
            # sync ring; vx (needed only after the first exp) on the scalar
            # ring, where it queues behind the dummy exp.
            tiles_in = [ip.tile([P, TW], bf16, name=f"in{j}") for j in range(n_units)]
            nc.sync.dma_start(tiles_in[0][:, 0:VX0], in_d[0][:, 0:VX0])
            nc.scalar.dma_start(tiles_in[0][:, VX0:TW], in_d[0][:, VX0:TW])

            for j in range(n_units):
                t = tiles_in[j]
                # prefetch the next unit's input ahead of this unit's store
                # on the in-order sync queue.
                if j + 1 < n_units:
                    nc.sync.dma_start(tiles_in[j + 1], in_d[j + 1])

                u_ps = pu.tile([P, S], f32)

                def mm1_pair(cp, t=t):
                    ks = slice(KT0 + cp * P, KT0 + (cp + 1) * P)
                    ta = pt.tile([P, S], f32, tag="tps", name=f"ta{cp}")
                    tb = pt.tile([P, S], f32, tag="tps", name=f"tb{cp}")
                    for h in range(NHALF):
                        qs = slice(h * 512, (h + 1) * 512)
                        nc.tensor.matmul(
                            ta[:, qs],
                            lhsT=t[0:D, ks],
                            rhs=t[0:D, qs],
                            start=True, stop=True,
                            tile_position=(0, 0),
                        )
                        nc.tensor.matmul(
                            tb[:, qs],
                            lhsT=t[D:P, ks],
                            rhs=t[D:P, qs],
                            start=True, stop=True,
                            tile_position=(64, 0),
                        )
                    return ta, tb

                def vxc(c, t=t):
                    return t[:, VX0 + c * P:VX0 + (c + 1) * P]

                # software-pipelined chunk loop: the next pair's mm1s are
                # emitted (adjacently, for row-group packing) before this
                # pair's exp+mm2 consumers.
                NP = NCHUNK // 2
                tiles = mm1_pair(0)
                for cp in range(NP):
                    nxt = mm1_pair(cp + 1) if cp + 1 < NP else None
                    for c, t_ps in zip((2 * cp, 2 * cp + 1), tiles):
                        e_sb = ep.tile([P, S], bf16)
                        nc.scalar.activation(
                            e_sb, t_ps, mybir.ActivationFunctionType.Exp,
                            bias=0.0, scale=0.125,
                        )
                        for h in range(NHALF):
                            qs = slice(h * 512, (h + 1) * 512)
                            nc.tensor.matmul(
                                u_ps[:, qs],
                                lhsT=vxc(c),
                                rhs=e_sb[:, qs],
                                start=(c == 0),
                                stop=(c == NCHUNK - 1),
                            )
                    tiles = nxt

                # U rows 0..63 hold out^T, row 64 the softmax denominator.
                # Copy U out of PSUM immediately (releases u_ps for the next
                # unit), then ship it; host does the division.  The last
                # unit streams in halves so the tail is copy/2 + dma/2.
                u_sb = rp.tile([D + 1, S], bf16)
                if j == n_units - 1:
                    # Split the last store so the halves issue on both HWDGE
                    # rings in parallel and the final transfer is short.
                    qs0, qs1 = slice(0, 512), slice(512, S)
                    nc.vector.tensor_copy(out=u_sb[:, qs0], in_=u_ps[0:D + 1, qs0])
                    nc.sync.dma_start(out_d[j][:, qs0], u_sb[:, qs0])
                    nc.vector.tensor_copy(out=u_sb[:, qs1], in_=u_ps[0:D + 1, qs1])
                    nc.scalar.dma_start(out_d[j][:, qs1], u_sb[:, qs1])
                else:
                    nc.vector.tensor_copy(out=u_sb, in_=u_ps[0:D + 1, :])
                    nc.sync.dma_start(out_d[j], u_sb)
    nc.compile()
    return nc


def _get_program(n_units):
    if n_units not in _program_cache:
        _program_cache[n_units] = _build_program(n_units)
    return _program_cache[n_units]


def _pack_unit(dst, qt, kt, vr):
    """Fill one packed [128, TW] bf16 slab from QT/KT [64, S] and V [S, 64]."""
    dst[0:D, QT0:QT0 + S] = qt
    dst[D:P, QT0:QT0 + S] = qt
    # kt packed: chunk pair p -> cols 128p:128(p+1), even chunk rows 0:64,
    # odd chunk rows 64:128
    kc = kt.reshape(D, NCHUNK, P)                         # [d, c, k]
    kp = dst[:, KT0:KT0 + 512].reshape(P, NCHUNK // 2, P)
    kp[0:D] = kc[:, 0::2]
    kp[D:P] = kc[:, 1::2]
    vx = dst[:, VX0:TW].reshape(P, NCHUNK, P)
    vx[:, :, 0:D] = vr.reshape(NCHUNK, P, D).transpose(1, 0, 2)
    vx[:, :, D:P] = np.float32(1.0)


def _prepare(Q, K, V, mask):
    """Host-side sharding. Returns (out_skeleton, units_per_core, in_maps)."""
    import ml_dtypes

    bf16 = ml_dtypes.bfloat16
    Q = np.ascontiguousarray(Q, dtype=np.float32)
    K = np.ascontiguousarray(K, dtype=np.float32)
    V = np.ascontiguousarray(V, dtype=np.float32)
    mask_b = np.asarray(mask).reshape(B).astype(bool)

    out = np.empty((B, H, S, D), dtype=np.float32)

    # Masked batches: softmax over a constant row is exactly uniform -> mean of V.
    for b in np.nonzero(mask_b)[0]:
        mv = V[b].mean(axis=1, dtype=np.float32)          # [H, D]
        out[b] = np.broadcast_to(mv[:, None, :], (H, S, D))

    units = [(b, h) for b in range(B) if not mask_b[b] for h in range(H)]
    if not units:
        return out, None, None

    # Pad to a multiple of NCORES with duplicates (identical redundant work).
    n_per = -(-len(units) // NCORES)
    padded = units + [units[0]] * (n_per * NCORES - len(units))
    per_core = [padded[i::NCORES] for i in range(NCORES)]

    QT = Q.transpose(0, 1, 3, 2).astype(bf16)             # [B,H,D,S]
    KT = K.transpose(0, 1, 3, 2).astype(bf16)
    Vr = V.astype(bf16)

    in_maps = []
    for core_units in per_core:
        inp = np.empty((len(core_units), P, TW), bf16)
        for s, (b, h) in enumerate(core_units):
            _pack_unit(inp[s], QT[b, h], KT[b, h], Vr[b, h])
        in_maps.append({"inp": inp})
    return out, per_core, in_maps


def _run_device(n_units, in_maps, trace=False, trace_cores=None):
    from concourse import bass_utils

    nc = _get_program(n_units)
    return bass_utils.run_bass_kernel_spmd(
        nc,
        in_maps,
        list(range(NCORES)),
        trace=trace,
        trace_cores=trace_cores,
    )


def kernel(Q, K, V, mask, _trace=False, _result_box=None):
    out, per_core, in_maps = _prepare(Q, K, V, mask)
    if in_maps is None:
        return out
    res = _run_device(len(per_core[0]), in_maps, trace=_trace)
    if _result_box is not None:
        _result_box.append(res)
    for i, core_units in enumerate(per_core):
        core_out = res.results[i]["out"]                  # [n, D+1, S]
        for s, (b, h) in enumerate(core_units):
            u = np.asarray(core_out[s], dtype=np.float32)
            out[b, h] = (u[0:D] / u[D:D + 1]).T
    return out
